# revision 1
# baseline (speedup 1.0000x reference)
"""Trainium2 Bass kernel for nn_Fast2Order_DE_Conv.

Math: out[b,o,ho,wo] = sum_{c,i,j} W[o, c*81+i*9+j] * p_i * p_j with
p_i = x[b, c, ho+di, wo+dj] (i = di*3+dj, 3x3 unfold of a 16-channel 64x64
image; output 62x62).

Algorithm: change the quadratic-feature basis from products p_i*p_j to
squares {p_i^2, (p_i+p_j)^2, i<j} (45 per channel, 720 total) and fold the
basis change into W on the host (W2 = W * M^-1).  On-chip, per spatial tile
of 512 locations:

    selection matmul (PE, f16):  s = AselT.T @ x_unfold  [768 padded rows]
    square          (ACT/DVE):   g = s^2, PSUM -> SBUF f16
    main matmul     (PE, f16):   out += W2T.T @ g, accumulated in fp32 PSUM

All matmuls use float16 (e5m10: ~f32r accuracy at half the width, 2-byte
FWL-eligible weight loads, full PE rate).  Inputs are cast to f16 on the
host so DMA loads feed the PE directly.  The 3x3 unfold itself is free: it
is expressed in the DMA access pattern (overlapping windows of the padded
l' = ho*64+wo layout).

Pipelining: tiles are software-pipelined with skew 3 (a tile's selection
matmuls + squares issue three tiles before its main matmuls) so the PE
never waits on the square engines; a burst of warmup matmuls during the
initial DMA window keeps the PE clock gate at full rate.  Per-core device
time ~55 us, ~80% of the PE streaming roofline; the residue is per-matmul
weight-load and dispatch overhead.

Sharding: data-parallel over batch, 2 batches per core on 8 cores; W-side
constants are replicated.  Output gathered by simple concatenation.
"""

import functools

import numpy as np

import concourse.bacc as bacc
import concourse.mybir as mybir
from concourse.tile import TileContext
from concourse.bass_utils import run_bass_kernel_spmd

B, C, H, WIDTH = 16, 16, 64, 64
O = 128
HO = WO = 62
N_CORES = 8
B_LOC = B // N_CORES
PAIRS = [(i, j) for i in range(9) for j in range(i, 9)]  # 45
ROW_TILES = [(0, 8), (8, 8), (16, 8), (24, 8), (32, 8), (40, 8), (48, 8), (56, 6)]
NCHUNK = 6  # g chunks of 128 rows (768 total, 48 zero-padded)
GC = 128
GH = 384  # padded g rows per c-half (360 real + 24 pad)


def _round_f32r(a: np.ndarray) -> np.ndarray:
    """Round fp32 values to the f32r grid (RNE at 12 low mantissa bits)."""
    a = np.ascontiguousarray(a, dtype=np.float32)
    bits = a.view(np.uint32).astype(np.uint64)
    half, mask = np.uint64(0x800), np.uint64(0xFFF)
    lsb = (bits >> np.uint64(12)) & np.uint64(1)
    out = ((bits + half - np.uint64(1) + lsb) & ~mask).astype(np.uint32)
    return out.view(np.float32).reshape(a.shape)


def _build_consts(Wf: np.ndarray):
    """W (128, 1296) -> (AselT [72, 360] f32, W2T [720, 128] f32, f32r grid)."""
    Wt = np.asarray(Wf, dtype=np.float64).reshape(O, C, 9, 9)
    Wsym = Wt + Wt.transpose(0, 1, 3, 2)
    W2 = np.zeros((O, 720))
    for c in range(C):
        for pi, (i, j) in enumerate(PAIRS):
            f = c * 45 + pi
            if i == j:
                W2[:, f] = Wt[:, c, i, i] - 0.5 * (
                    Wsym[:, c, i, :].sum(-1) - 2.0 * Wt[:, c, i, i]
                )
            else:
                W2[:, f] = 0.5 * Wsym[:, c, i, j]
    # x-row layout on chip: row = i*8 + c_local (i = di*3+dj kernel position)
    AselT = np.zeros((72, 384), dtype=np.float32)
    for cl in range(8):
        for pi, (i, j) in enumerate(PAIRS):
            g = cl * 45 + pi
            AselT[i * 8 + cl, g] += 1.0
            if i != j:
                AselT[j * 8 + cl, g] += 1.0
    # pad each c-half's 360 features to 384 (3 chunks of 128) so every
    # selection matmul has exactly 128 stationary columns (enables FWL)
    W2p = np.zeros((O, 768))
    W2p[:, 0:360] = W2[:, 0:360]
    W2p[:, 384:744] = W2[:, 360:720]
    W2T = np.ascontiguousarray(W2p.T).astype(np.float16)  # [768, 128]
    return AselT.astype(np.float16), W2T


def _x_window_ap(x_d, b: int, h: int, ho0: int, di: int, lt_load: int):
    """Source AP for one di of the unfold load: (dj, c, l) nesting matching
    target partitions (di*3+dj)*8 + c, free dim = padded l' = ho*64+wo."""
    ap = x_d[b, h * 8 : (h + 1) * 8, ho0 + di, 0:3].unsqueeze(-1)
    v = ap.ap
    v[0] = [1, 3]
    v[1] = [H * WIDTH, 8]
    v[2] = [1, lt_load]
    return ap


def build_nc(reps: int = 1, skew: int = 3):
    """Build the per-core program.  reps>1 wraps the body in an on-chip loop
    (used only for device-time measurement); skew is the software-pipeline
    depth between a tile's selection/squares and its main matmuls."""
    f32, f16 = mybir.dt.float32, mybir.dt.float16
    nc = bacc.Bacc("TRN2", target_bir_lowering=False)
    x_d = nc.dram_tensor("x_loc", [B_LOC, C, H, WIDTH], f16, kind="ExternalInput")
    a_d = nc.dram_tensor("aselT", [72, GH], f16, kind="ExternalInput")
    w_d = nc.dram_tensor("w2T", [2 * GH, O], f16, kind="ExternalInput")
    o_d = nc.dram_tensor("out_loc", [B_LOC, O, HO, WO], f32, kind="ExternalOutput")

    with TileContext(nc) as tc:
        with (
            tc.tile_pool(name="const", bufs=1) as cpool,
            tc.tile_pool(name="xin", bufs=2) as xpool,
            tc.tile_pool(name="gbuf", bufs=3 * (skew + 1) + 3) as gpool,
            tc.tile_pool(name="tmpbuf", bufs=4) as tmppool,
            tc.tile_pool(name="obuf", bufs=6) as opool,
            tc.tile_pool(name="ps_sel", bufs=3, space="PSUM") as pspool,
            tc.tile_pool(name="ps_out", bufs=2, space="PSUM") as popool,
        ):
            LFULL = HO * 64  # 3968 columns of the padded l' = ho*64+wo axis

            a_r = cpool.tile([72, GH], f16, tag="a_r")
            nc.sync.dma_start(a_r[:], a_d[:])

            def load_x(x_t, b, h, col0, col1, eng=None):
                """Fill x_t[:, col0:col1] of the unfold view for (b, c-half h)."""
                eng = eng or nc.sync
                for di in range(3):
                    hi = min(col1, H * WIDTH - di * 64 - 2)
                    if hi > col0:
                        ap = _x_window_ap(x_d, b, h, 0, di, hi - col0)
                        ap.offset += col0
                        eng.dma_start(x_t[di * 24 : (di + 1) * 24, col0:hi], ap)
                    if hi < col1:
                        # pad columns feed discarded outputs; fill with
                        # arbitrary valid f32r data to keep reads clean
                        eng.dma_start(
                            x_t[di * 24 : (di + 1) * 24, hi:col1],
                            _x_window_ap(x_d, b, h, 0, 0, col1 - hi),
                        )

            # all unfold loads up front; batch 0 split so tile 0 starts early
            xr_all = []
            for b in range(B_LOC):
                xr_b = []
                for h in range(2):
                    x_t = xpool.tile([72, LFULL], f16, tag=f"x{h}", name=f"x{h}_{b}")
                    xr_b.append(x_t)
                xr_all.append(xr_b)
            for h in range(2):
                load_x(xr_all[0][h], 0, h, 0, 1024)
            w_r = cpool.tile([GC, NCHUNK, O], f16, tag="w_r")
            nc.sync.dma_start(w_r[:], w_d[:].rearrange("(k p) o -> p k o", p=GC))
            for h in range(2):
                load_x(xr_all[0][h], 0, h, 1024, LFULL)
            for b in range(1, B_LOC):
                for h in range(2):
                    load_x(xr_all[b][h], b, h, 0, LFULL)

            # greedy ACT/DVE load balancing for PSUM-draining elementwise
            # ops (DVE pays double for squares: bounce + SBUF square)
            eng_busy = {"act": 0.0, "dve": 0.0}

            def square_merged(g_t, ps_s, lt):
                gv = g_t[:, :, :lt]
                pv = ps_s[:, :, :lt]
                if eng_busy["act"] + 1.0 <= eng_busy["dve"] + 2.1:
                    nc.scalar.square(gv, pv)
                    eng_busy["act"] += 1.0
                else:
                    tmp = tmppool.tile([GC, 2, 512], f32, tag="sq_tmp")
                    tv = tmp[:, :, :lt]
                    nc.vector.tensor_copy(tv, pv)
                    nc.vector.tensor_mul(gv, tv, tv)
                    eng_busy["dve"] += 2.1

            def out_copy(o_view, ps_view):
                if eng_busy["act"] + 0.9 < eng_busy["dve"] + 0.55:
                    nc.scalar.copy(o_view, ps_view)
                    eng_busy["act"] += 0.9
                else:
                    nc.vector.tensor_copy(o_view, ps_view)
                    eng_busy["dve"] += 0.55

            def do_mains(st):
                """Main matmuls + drain for a tile whose squares are issued."""
                b, ho0, nr, g_ts = st
                lt = nr * 64
                ps_o = popool.tile([O, 512], f32, tag="ps_o", name="ps_o")
                for kk in range(NCHUNK):
                    nc.tensor.matmul(
                        ps_o[:, :lt],
                        w_r[:, kk, :],
                        g_ts[kk // 2][:, kk % 2, :lt],
                        start=(kk == 0),
                        stop=(kk == NCHUNK - 1),
                    )
                # compact to [O, nr*62] so the store uses contiguous chunks
                o_t = opool.tile([O, 8 * WO], f32, tag="o", name="o_t")
                ps_view = ps_o[:, :lt].rearrange("o (r w) -> o r w", w=64)
                o_view = o_t[:, : nr * WO].rearrange("o (r w) -> o r w", w=WO)
                out_copy(o_view, ps_view[:, :, :WO])
                nc.gpsimd.dma_start(
                    o_d[b, :, ho0 : ho0 + nr, :],
                    o_t[:, : nr * WO],
                )

            # HAM warmup: keep the PE busy during the initial DMA wait so the
            # clock gate is at 8/8 when real matmuls start (dummy MMs on the
            # first tile that lands; outputs never read)
            def warmup():
                for i in range(12):
                    ps_w = popool.tile([O, 512], f32, tag="ps_o", name="warm")
                    nc.tensor.matmul(
                        ps_w[:, :360], a_r[:, :128], a_r[:, :360],
                        start=True, stop=True,
                    )

            def body(it=None, unroll=1):
                # software-pipeline skew: issue tile t's selections and
                # squares, then tile (t-skew)'s mains — squares get `skew`
                # tiles of slack before the PE needs their output
                pending = []
                for b in range(B_LOC):
                    xr = xr_all[b]
                    for ho0, nr in ROW_TILES:
                        lt = nr * 64
                        c0 = ho0 * 64
                        g_ts = []
                        for kp in range(NCHUNK // 2):
                            # two 120-row chunks share one 2-bank PSUM tile so
                            # one elementwise op drains both
                            ps_s = pspool.tile(
                                [GC, 2, 512], f32, tag="ps_s", name="ps_s"
                            )
                            for half in range(2):
                                kk = kp * 2 + half
                                h, k = divmod(kk, 3)
                                nc.tensor.matmul(
                                    ps_s[:, half, :lt],
                                    a_r[:, k * GC : (k + 1) * GC],
                                    xr[h][:, c0 : c0 + lt],
                                    start=True,
                                    stop=True,
                                )
                            g_t = gpool.tile(
                                [GC, 2, 512], f16, tag="g", name="g_t"
                            )
                            square_merged(g_t, ps_s, lt)
                            g_ts.append(g_t)
                        pending.append((b, ho0, nr, g_ts))
                        if len(pending) > skew:
                            do_mains(pending.pop(0))
                for st in pending:
                    do_mains(st)

            warmup()
            if reps == 1:
                body()
            else:
                hint = (
                    mybir.EngineType.PE,
                    mybir.EngineType.Activation,
                    mybir.EngineType.DVE,
                    mybir.EngineType.SP,
                    mybir.EngineType.Pool,
                )
                with tc.For_i(0, reps, 1, hint_engines=hint) as _it:
                    body()
    nc.compile()
    return nc


@functools.lru_cache(maxsize=1)
def _cached_nc():
    return build_nc()


def kernel(x: np.ndarray, W: np.ndarray, _trace: bool = False):
    x = np.asarray(x, dtype=np.float32)
    W = np.asarray(W, dtype=np.float32)
    AselT, W2T = _build_consts(W)
    x_r = x.astype(np.float16)

    nc = _cached_nc()
    in_maps = [
        {
            "x_loc": np.ascontiguousarray(x_r[k * B_LOC : (k + 1) * B_LOC]),
            "aselT": AselT,
            "w2T": W2T,
        }
        for k in range(N_CORES)
    ]
    try:
        r = run_bass_kernel_spmd(
            nc, in_maps, core_ids=list(range(N_CORES)), trace=_trace
        )
    except Exception:
        # transient NRT_EXEC_UNIT_UNRECOVERABLE has been observed once on
        # this fabric; a fresh attempt recovers
        r = run_bass_kernel_spmd(
            nc, in_maps, core_ids=list(range(N_CORES)), trace=_trace
        )
    out = np.concatenate([m["out_loc"] for m in r.results], axis=0)
    if _trace:
        kernel.last_result = r
    return out


if __name__ == "__main__":
    rng = np.random.default_rng(0)
    x = rng.standard_normal((B, C, H, WIDTH), dtype=np.float32)
    W = rng.standard_normal((O, C * 81), dtype=np.float32)
    out = kernel(x, W)
    print("out shape", out.shape, out.dtype)



# revision 32
# speedup vs baseline: 4.2001x; 4.2001x over previous
"""Trainium2 Bass kernel for nn_Fast2Order_DE_Conv.

Math: out[b,o,ho,wo] = sum_{c,i,j} W[o, c*81+i*9+j] * p_i * p_j with
p_i = x[b, c, ho+di, wo+dj] (i = di*3+dj, 3x3 unfold of a 16-channel 64x64
image; output 62x62).

Algorithm: change the quadratic-feature basis from products p_i*p_j to
squares {p_i^2, (p_i+p_j)^2, i<j} (45 per channel, 720 total) and fold the
basis change into W on the host (W2 = W * M^-1).  On-chip, per spatial tile
of 512 locations:

    selection matmul (PE, f16):  s = AselT.T @ x_unfold  [768 padded rows]
    square          (ACT/DVE):   g = s^2, PSUM -> SBUF f16
    main matmul     (PE, f16):   out += W2T.T @ g, accumulated in fp32 PSUM

All matmuls use float16 (e5m10: ~f32r accuracy at half the width, 2-byte
FWL-eligible weight loads, full PE rate).  Inputs are cast to f16 on the
host so DMA loads feed the PE directly.  The 3x3 unfold itself is free: it
is expressed in the DMA access pattern (overlapping windows of the padded
l' = ho*64+wo layout).

Pipelining: tiles are software-pipelined with skew 2 (a tile's selection
matmuls + squares issue two tiles before its main matmuls) so the PE
rarely waits on the square engines; a burst of warmup matmuls during the
initial DMA window keeps the PE clock gate at full rate.

DVE squares bounce PSUM->SBUF through an f16 tmp so the multiply runs in
the DVE 2x_1p perf mode; ACT squares go straight from PSUM.  A greedy
balancer splits squares and output copies across ACT/DVE by modeled ns.
Output stores alternate between the Pool SWDGE and SP HWDGE queues so
consecutive stores issue in parallel, and the final tile's PSUM drain is
split across both elementwise engines to shorten the kernel tail.  A
post-build pass drops Ldweights whose stationary operand is already
loaded (warmup runs and back-to-back same-weight matmuls).

Sharding: data-parallel over batch, 2 batches per core on 8 cores; W-side
constants are replicated.  Output gathered by simple concatenation.
"""

import functools

import numpy as np

import concourse.bacc as bacc
import concourse.mybir as mybir
from concourse.tile import TileContext
from concourse.bass_utils import run_bass_kernel_spmd

B, C, H, WIDTH = 16, 16, 64, 64
O = 128
HO = WO = 62
N_CORES = 8
B_LOC = B // N_CORES
PAIRS = [(i, j) for i in range(9) for j in range(i, 9)]  # 45
ROW_TILES = [(0, 8), (8, 8), (16, 8), (24, 8), (32, 8), (40, 8), (48, 8), (56, 6)]
NCHUNK = 6  # g chunks of 128 rows (768 total, 48 zero-padded)
GC = 128
GH = 384  # padded g rows per c-half (360 real + 24 pad)


def _round_f32r(a: np.ndarray) -> np.ndarray:
    """Round fp32 values to the f32r grid (RNE at 12 low mantissa bits)."""
    a = np.ascontiguousarray(a, dtype=np.float32)
    bits = a.view(np.uint32).astype(np.uint64)
    half, mask = np.uint64(0x800), np.uint64(0xFFF)
    lsb = (bits >> np.uint64(12)) & np.uint64(1)
    out = ((bits + half - np.uint64(1) + lsb) & ~mask).astype(np.uint32)
    return out.view(np.float32).reshape(a.shape)


def _build_consts(Wf: np.ndarray):
    """W (128, 1296) -> (AselT [72, 384] f16, W2T [768, 128] f16)."""
    Wt = np.asarray(Wf, dtype=np.float64).reshape(O, C, 9, 9)
    Wsym = Wt + Wt.transpose(0, 1, 3, 2)
    W2 = np.zeros((O, 720))
    for c in range(C):
        for pi, (i, j) in enumerate(PAIRS):
            f = c * 45 + pi
            if i == j:
                W2[:, f] = Wt[:, c, i, i] - 0.5 * (
                    Wsym[:, c, i, :].sum(-1) - 2.0 * Wt[:, c, i, i]
                )
            else:
                W2[:, f] = 0.5 * Wsym[:, c, i, j]
    # x-row layout on chip: row = i*8 + c_local (i = di*3+dj kernel position)
    AselT = np.zeros((72, 384), dtype=np.float32)
    for cl in range(8):
        for pi, (i, j) in enumerate(PAIRS):
            g = cl * 45 + pi
            AselT[i * 8 + cl, g] += 1.0
            if i != j:
                AselT[j * 8 + cl, g] += 1.0
    # pad each c-half's 360 features to 384 (3 chunks of 128) so every
    # selection matmul has exactly 128 stationary columns
    W2p = np.zeros((O, 768))
    W2p[:, 0:360] = W2[:, 0:360]
    W2p[:, 384:744] = W2[:, 360:720]
    W2T = np.ascontiguousarray(W2p.T).astype(np.float16)  # [768, 128]
    return AselT.astype(np.float16), W2T


def _x_window_ap(x_d, b: int, h: int, ho0: int, di: int, lt_load: int):
    """Source AP for one di of the unfold load: (dj, c, l) nesting matching
    target partitions (di*3+dj)*8 + c, free dim = padded l' = ho*64+wo."""
    ap = x_d[b, h * 8 : (h + 1) * 8, ho0 + di, 0:3].unsqueeze(-1)
    v = ap.ap
    v[0] = [1, 3]
    v[1] = [H * WIDTH, 8]
    v[2] = [1, lt_load]
    return ap


def _ldw_signature(inst):
    """Identity of a Ldweights' stationary operand."""
    return str(inst.ins[0])


def _dedupe_ldweights(nc):
    """Remove Ldweights whose stationary operand is identical to the still-
    loaded one (only Matmults / non-PE instructions in between).  Waits and
    updates of a removed Ldweights move to the following PE instruction."""
    pe = mybir.EngineType.PE
    removed = 0
    for blk in nc.m.functions[0].blocks:
        last_sig = None
        insts = blk.instructions
        idx = 0
        while idx < len(insts):
            inst = insts[idx]
            if getattr(inst, "engine", None) != pe:
                idx += 1
                continue
            op = str(inst.opcode)
            if op == "Ldweights":
                sig = _ldw_signature(inst)
                if sig is not None and sig == last_sig:
                    nxt = None
                    for j in range(idx + 1, len(insts)):
                        if getattr(insts[j], "engine", None) == pe:
                            nxt = insts[j]
                            break
                    si = inst.sync_info
                    has_sync = si is not None and (
                        len(si.on_wait) > 0 or len(si.on_update) > 0
                    )
                    if nxt is not None:
                        nxt.merge_dependencies_from(inst)
                        if has_sync:
                            nsi = nxt.sync_info
                            if nsi is None:
                                nxt.sync_info = si
                            else:
                                for w in si.on_wait:
                                    nsi.on_wait.append(w)
                                for u in si.on_update:
                                    nsi.on_update.append(u)
                        del insts[idx]
                        removed += 1
                        continue
                last_sig = sig
            elif op != "Matmult":
                if op not in ("EventSemaphore", "Nop"):
                    last_sig = None
            idx += 1
    return removed


def build_nc(
    reps: int = 1,
    skew: int = 2,
    py_reps: int = 1,
    dedupe: bool = True,
    sel_group: int = 2,
    tail_split: bool = True,
    defer_copy: bool = False,
    alt_store: bool = True,
    end_taper: int = 0,
    compact: bool = False,
    alt_drains: bool = False,
    dve_w: float = 1.79,
):
    """Build the per-core program.  reps>1 wraps the body in an on-chip loop
    (used only for device-time measurement); skew is the software-pipeline
    depth between a tile's selection/squares and its main matmuls;
    sel_group is the number of PSUM banks drained by one square op."""
    f32, f16 = mybir.dt.float32, mybir.dt.float16
    nc = bacc.Bacc("TRN2", target_bir_lowering=False)
    x_d = nc.dram_tensor("x_loc", [B_LOC, C, H, WIDTH], f16, kind="ExternalInput")
    a_d = nc.dram_tensor("aselT", [72, GH], f16, kind="ExternalInput")
    w_d = nc.dram_tensor("w2T", [2 * GH, O], f16, kind="ExternalInput")
    o_d = nc.dram_tensor("out_loc", [B_LOC, O, HO, WO], f32, kind="ExternalOutput")

    ndrain = NCHUNK // sel_group  # square ops per tile
    sel_bufs = 6 // sel_group  # keep 6 PSUM banks for selections

    with TileContext(nc) as tc:
        with (
            tc.tile_pool(name="const", bufs=1) as cpool,
            tc.tile_pool(name="xin", bufs=2) as xpool,
            tc.tile_pool(name="gbuf", bufs=ndrain * (skew + 1) + ndrain) as gpool,
            tc.tile_pool(name="tmpbuf", bufs=4) as tmppool,
            tc.tile_pool(name="obuf", bufs=6) as opool,
            tc.tile_pool(name="ps_sel", bufs=sel_bufs, space="PSUM") as pspool,
            tc.tile_pool(name="ps_out", bufs=2, space="PSUM") as popool,
        ):
            if compact:
                LFULL = HO * WO  # 3844 dense l = ho*62+wo columns
                TILES = [(k * 512, 512) for k in range(7)] + [(3584, 260)]
            else:
                LFULL = HO * 64  # 3968 columns of the padded l' layout
                TILES = [(ho0 * 64, nr * 64) for ho0, nr in ROW_TILES]

            a_r = cpool.tile([72, GH], f16, tag="a_r")
            nc.sync.dma_start(a_r[:], a_d[:])

            def load_x(x_t, b, h, col0, col1, eng=None):
                """Fill x_t[:, col0:col1] of the unfold view for (b, c-half h)."""
                eng = eng or nc.sync
                for di in range(3):
                    hi = min(col1, H * WIDTH - di * 64 - 2)
                    if hi > col0:
                        ap = _x_window_ap(x_d, b, h, 0, di, hi - col0)
                        ap.offset += col0
                        eng.dma_start(x_t[di * 24 : (di + 1) * 24, col0:hi], ap)
                    if hi < col1:
                        # pad columns feed discarded outputs; fill with
                        # arbitrary valid f32r data to keep reads clean
                        eng.dma_start(
                            x_t[di * 24 : (di + 1) * 24, hi:col1],
                            _x_window_ap(x_d, b, h, 0, 0, col1 - hi),
                        )

            def load_x_compact(x_t, b, h, ho0, ho1, eng=None):
                """Fill x_t[:, ho0*62:ho1*62] of the dense unfold view.
                One DMA per kernel offset (di, dj): 3-dim APs."""
                eng = eng or nc.sync
                nrow = ho1 - ho0
                for di in range(3):
                    for dj in range(3):
                        ap = x_d[b, h * 8 : (h + 1) * 8, di, dj : dj + 1]
                        ap = ap.unsqueeze(-1)
                        v = ap.ap
                        v[0] = [H * WIDTH, 8]  # channel
                        v[1] = [WIDTH, nrow]   # ho rows
                        v[2] = [1, WO]         # wo
                        ap.offset += ho0 * WIDTH
                        r0 = (di * 3 + dj) * 8
                        eng.dma_start(
                            x_t[r0 : r0 + 8, ho0 * WO : ho1 * WO], ap
                        )

            # all unfold loads up front; batch 0 split so tile 0 starts early
            xr_all = []
            for b in range(B_LOC):
                xr_b = []
                for h in range(2):
                    x_t = xpool.tile([72, LFULL], f16, tag=f"x{h}", name=f"x{h}_{b}")
                    xr_b.append(x_t)
                xr_all.append(xr_b)
            if compact:
                for h in range(2):
                    load_x_compact(xr_all[0][h], 0, h, 0, 17)
            else:
                for h in range(2):
                    load_x(xr_all[0][h], 0, h, 0, 1024)
            w_r = cpool.tile([GC, NCHUNK, O], f16, tag="w_r")
            nc.sync.dma_start(w_r[:], w_d[:].rearrange("(k p) o -> p k o", p=GC))
            if compact:
                for h in range(2):
                    load_x_compact(xr_all[0][h], 0, h, 17, HO)
                for b in range(1, B_LOC):
                    for h in range(2):
                        load_x_compact(xr_all[b][h], b, h, 0, HO)
            else:
                for h in range(2):
                    load_x(xr_all[0][h], 0, h, 1024, LFULL)
                for b in range(1, B_LOC):
                    for h in range(2):
                        load_x(xr_all[b][h], b, h, 0, LFULL)

            # greedy ACT/DVE load balancing for PSUM-draining elementwise
            # ops; weights are modeled engine-ns for the op shapes in use
            eng_busy = {"act": 0.0, "dve": 0.0}
            SQ_W = {2: (1.04, dve_w), 3: (1.47, 2.59)}[sel_group]

            def square_merged(g_t, ps_s, lt, kp=None):
                gv = g_t[:, :, :lt]
                pv = ps_s[:, :, :lt]
                if alt_drains and kp is not None and kp < 2:
                    use_act = kp == 0  # first drain fast-latency on ACT
                else:
                    use_act = eng_busy["act"] + SQ_W[0] <= eng_busy["dve"] + SQ_W[1]
                if use_act:
                    nc.scalar.square(gv, pv)
                    eng_busy["act"] += SQ_W[0]
                else:
                    tmp = tmppool.tile([GC, sel_group, 512], f16, tag="sq_tmp")
                    tv = tmp[:, :, :lt]
                    nc.vector.tensor_copy(tv, pv)
                    nc.vector.tensor_mul(gv, tv, tv)
                    eng_busy["dve"] += SQ_W[1]

            def out_copy(o_view, ps_view, force=None):
                if force == "act" or (
                    force is None
                    and eng_busy["act"] + 0.60 < eng_busy["dve"] + 0.64
                ):
                    nc.scalar.copy(o_view, ps_view)
                    eng_busy["act"] += 0.60
                else:
                    nc.vector.tensor_copy(o_view, ps_view)
                    eng_busy["dve"] += 0.64

            copy_queue = []
            store_ctr = {"n": 0}

            def _out_flat_ap(b, c0, lt):
                """Flat [O, lt] view of out_loc at dense column offset c0."""
                ap = o_d[b, :, 0, 0:2].unsqueeze(-1)
                v = ap.ap
                v[1] = [1, lt]
                v[2] = [1, 1]
                ap.offset += c0
                return ap

            def drain_tile(item, last=False):
                """Drain one accumulator to SBUF and store it."""
                b, c0, lt, ps_o = item
                eng = nc.gpsimd
                if alt_store:
                    store_ctr["n"] += 1
                    if store_ctr["n"] % 2 == 0 or last:
                        eng = nc.sync
                if compact:
                    o_t = opool.tile([O, 512], f32, tag="o", name="o_t")
                    if last and tail_split and lt >= 128:
                        half = lt // 2
                        out_copy(o_t[:, :half], ps_o[:, :half], force="act")
                        out_copy(o_t[:, half:lt], ps_o[:, half:lt], force="dve")
                    else:
                        out_copy(o_t[:, :lt], ps_o[:, :lt])
                    eng.dma_start(_out_flat_ap(b, c0, lt), o_t[:, :lt])
                else:
                    ho0, nr = c0 // 64, lt // 64
                    o_t = opool.tile([O, 8 * WO], f32, tag="o", name="o_t")
                    ps_view = ps_o[:, :lt].rearrange("o (r w) -> o r w", w=64)
                    o_view = o_t[:, : nr * WO].rearrange("o (r w) -> o r w", w=WO)
                    if last and tail_split and nr > 1:
                        half = nr // 2
                        out_copy(o_view[:, :half], ps_view[:, :half, :WO], force="act")
                        out_copy(o_view[:, half:], ps_view[:, half:nr, :WO], force="dve")
                    else:
                        out_copy(o_view, ps_view[:, :, :WO])
                    eng.dma_start(
                        o_d[b, :, ho0 : ho0 + nr, :],
                        o_t[:, : nr * WO],
                    )

            def do_mains(st, last=False):
                """Main matmuls + drain for a tile whose squares are issued."""
                b, c0, lt, g_ts = st
                ps_o = popool.tile([O, 512], f32, tag="ps_o", name="ps_o")
                for kk in range(NCHUNK):
                    nc.tensor.matmul(
                        ps_o[:, :lt],
                        w_r[:, kk, :],
                        g_ts[kk // sel_group][:, kk % sel_group, :lt],
                        start=(kk == 0),
                        stop=(kk == NCHUNK - 1),
                    )
                if defer_copy:
                    # emit the PSUM->SBUF drain one tile-slot later so it
                    # enqueues behind the next tile's PE-blocking squares
                    copy_queue.append((b, c0, lt, ps_o))
                    if len(copy_queue) > 1:
                        drain_tile(copy_queue.pop(0))
                    if last:
                        while copy_queue:
                            drain_tile(copy_queue.pop(0), last=not copy_queue)
                else:
                    drain_tile((b, c0, lt, ps_o), last=last)

            # HAM warmup: keep the PE busy during the initial DMA wait so the
            # clock gate is at 8/8 when real matmuls start (dummy MMs on the
            # first tile that lands; outputs never read)
            def warmup():
                for i in range(12):
                    ps_w = popool.tile([O, 512], f32, tag="ps_o", name="warm")
                    nc.tensor.matmul(
                        ps_w[:, :360], a_r[:, :128], a_r[:, :360],
                        start=True, stop=True,
                    )

            def body(it=None, unroll=1):
                # software-pipeline skew: issue tile t's selections and
                # squares, then tile (t-skew)'s mains — squares get `skew`
                # tiles of slack before the PE needs their output
                pending = []
                tiles_total = B_LOC * len(TILES)
                for b in range(B_LOC):
                    xr = xr_all[b]
                    for ti, (c0, lt) in enumerate(TILES):
                        g_ts = []
                        for kp in range(ndrain):
                            ps_s = pspool.tile(
                                [GC, sel_group, 512], f32, tag="ps_s", name="ps_s"
                            )
                            for half in range(sel_group):
                                kk = kp * sel_group + half
                                h, k = divmod(kk, 3)
                                nc.tensor.matmul(
                                    ps_s[:, half, :lt],
                                    a_r[:, k * GC : (k + 1) * GC],
                                    xr[h][:, c0 : c0 + lt],
                                    start=True,
                                    stop=True,
                                )
                            g_t = gpool.tile(
                                [GC, sel_group, 512], f16, tag="g", name="g_t"
                            )
                            square_merged(g_t, ps_s, lt, kp=kp)
                            g_ts.append(g_t)
                        pending.append((b, c0, lt, g_ts))
                        tile_idx = b * len(TILES) + ti
                        eff_skew = skew
                        if end_taper and tile_idx >= tiles_total - end_taper:
                            eff_skew = max(1, skew - (tile_idx - (tiles_total - end_taper) + 1))
                        if len(pending) > eff_skew:
                            do_mains(pending.pop(0))
                for i, st in enumerate(pending):
                    do_mains(st, last=(i == len(pending) - 1))

            warmup()
            if py_reps > 1:
                # python-unrolled loop for TimelineSim steady-state modeling
                for _ in range(py_reps):
                    body()
            elif reps == 1:
                body()
            else:
                hint = (
                    mybir.EngineType.PE,
                    mybir.EngineType.Activation,
                    mybir.EngineType.DVE,
                    mybir.EngineType.SP,
                    mybir.EngineType.Pool,
                )
                with tc.For_i(0, reps, 1, hint_engines=hint) as _it:
                    body()
    if dedupe:
        _dedupe_ldweights(nc)
    nc.compile()
    return nc


@functools.lru_cache(maxsize=1)
def _cached_nc():
    return build_nc()


def kernel(x: np.ndarray, W: np.ndarray, _trace: bool = False):
    x = np.asarray(x, dtype=np.float32)
    W = np.asarray(W, dtype=np.float32)
    AselT, W2T = _build_consts(W)
    x_r = x.astype(np.float16)

    nc = _cached_nc()
    in_maps = [
        {
            "x_loc": np.ascontiguousarray(x_r[k * B_LOC : (k + 1) * B_LOC]),
            "aselT": AselT,
            "w2T": W2T,
        }
        for k in range(N_CORES)
    ]
    try:
        r = run_bass_kernel_spmd(
            nc, in_maps, core_ids=list(range(N_CORES)), trace=_trace
        )
    except Exception:
        # transient NRT_EXEC_UNIT_UNRECOVERABLE has been observed once on
        # this fabric; a fresh attempt recovers
        r = run_bass_kernel_spmd(
            nc, in_maps, core_ids=list(range(N_CORES)), trace=_trace
        )
    out = np.concatenate([m["out_loc"] for m in r.results], axis=0)
    if _trace:
        kernel.last_result = r
    return out


if __name__ == "__main__":
    rng = np.random.default_rng(0)
    x = rng.standard_normal((B, C, H, WIDTH), dtype=np.float32)
    W = rng.standard_normal((O, C * 81), dtype=np.float32)
    out = kernel(x, W)
    print("out shape", out.shape, out.dtype)


# revision 37
# speedup vs baseline: 4.2023x; 1.0005x over previous
"""Trainium2 Bass kernel for nn_Fast2Order_DE_Conv.

Math: out[b,o,ho,wo] = sum_{c,i,j} W[o, c*81+i*9+j] * p_i * p_j with
p_i = x[b, c, ho+di, wo+dj] (i = di*3+dj, 3x3 unfold of a 16-channel 64x64
image; output 62x62).

Algorithm: change the quadratic-feature basis from products p_i*p_j to
squares {p_i^2, (p_i+p_j)^2, i<j} (45 per channel, 720 total) and fold the
basis change into W on the host (W2 = W * M^-1).  On-chip, per spatial tile
of 512 locations:

    selection matmul (PE, f16):  s = AselT.T @ x_unfold  [768 padded rows]
    square          (ACT/DVE):   g = s^2, PSUM -> SBUF f16
    main matmul     (PE, f16):   out += W2T.T @ g, accumulated in fp32 PSUM

All matmuls use float16 (e5m10: ~f32r accuracy at half the width, 2-byte
FWL-eligible weight loads, full PE rate).  Inputs are cast to f16 on the
host so DMA loads feed the PE directly.  The 3x3 unfold itself is free: it
is expressed in the DMA access pattern (overlapping windows of the padded
l' = ho*64+wo layout).

Pipelining: tiles are software-pipelined with skew 1 (a tile's selection
matmuls + squares issue one tile before its main matmuls; the shallow skew
with deep tmp/output buffering measured fastest on this fabric); a burst
of warmup matmuls during the initial DMA window keeps the PE clock gate at
full rate.

DVE squares bounce PSUM->SBUF through an f16 tmp so the multiply runs in
the DVE 2x_1p perf mode; ACT squares go straight from PSUM.  A greedy
balancer splits squares and output copies across ACT/DVE by modeled ns.
Output stores alternate between the Pool SWDGE and SP HWDGE queues so
consecutive stores issue in parallel, and the final tile's PSUM drain is
split across both elementwise engines to shorten the kernel tail.  A
post-build pass drops Ldweights whose stationary operand is already
loaded (warmup runs and back-to-back same-weight matmuls).

Sharding: data-parallel over batch, 2 batches per core on 8 cores; W-side
constants are replicated.  Output gathered by simple concatenation.
"""

import functools

import numpy as np

import concourse.bacc as bacc
import concourse.mybir as mybir
from concourse.tile import TileContext
from concourse.bass_utils import run_bass_kernel_spmd

B, C, H, WIDTH = 16, 16, 64, 64
O = 128
HO = WO = 62
N_CORES = 8
B_LOC = B // N_CORES
PAIRS = [(i, j) for i in range(9) for j in range(i, 9)]  # 45
ROW_TILES = [(0, 8), (8, 8), (16, 8), (24, 8), (32, 8), (40, 8), (48, 8), (56, 6)]
NCHUNK = 6  # g chunks of 128 rows (768 total, 48 zero-padded)
GC = 128
GH = 384  # padded g rows per c-half (360 real + 24 pad)


def _round_f32r(a: np.ndarray) -> np.ndarray:
    """Round fp32 values to the f32r grid (RNE at 12 low mantissa bits)."""
    a = np.ascontiguousarray(a, dtype=np.float32)
    bits = a.view(np.uint32).astype(np.uint64)
    half, mask = np.uint64(0x800), np.uint64(0xFFF)
    lsb = (bits >> np.uint64(12)) & np.uint64(1)
    out = ((bits + half - np.uint64(1) + lsb) & ~mask).astype(np.uint32)
    return out.view(np.float32).reshape(a.shape)


def _build_consts(Wf: np.ndarray):
    """W (128, 1296) -> (AselT [72, 384] f16, W2T [768, 128] f16)."""
    Wt = np.asarray(Wf, dtype=np.float64).reshape(O, C, 9, 9)
    Wsym = Wt + Wt.transpose(0, 1, 3, 2)
    W2 = np.zeros((O, 720))
    for c in range(C):
        for pi, (i, j) in enumerate(PAIRS):
            f = c * 45 + pi
            if i == j:
                W2[:, f] = Wt[:, c, i, i] - 0.5 * (
                    Wsym[:, c, i, :].sum(-1) - 2.0 * Wt[:, c, i, i]
                )
            else:
                W2[:, f] = 0.5 * Wsym[:, c, i, j]
    # x-row layout on chip: row = i*8 + c_local (i = di*3+dj kernel position)
    AselT = np.zeros((72, 384), dtype=np.float32)
    for cl in range(8):
        for pi, (i, j) in enumerate(PAIRS):
            g = cl * 45 + pi
            AselT[i * 8 + cl, g] += 1.0
            if i != j:
                AselT[j * 8 + cl, g] += 1.0
    # pad each c-half's 360 features to 384 (3 chunks of 128) so every
    # selection matmul has exactly 128 stationary columns
    W2p = np.zeros((O, 768))
    W2p[:, 0:360] = W2[:, 0:360]
    W2p[:, 384:744] = W2[:, 360:720]
    W2T = np.ascontiguousarray(W2p.T).astype(np.float16)  # [768, 128]
    return AselT.astype(np.float16), W2T


def _x_window_ap(x_d, b: int, h: int, ho0: int, di: int, lt_load: int):
    """Source AP for one di of the unfold load: (dj, c, l) nesting matching
    target partitions (di*3+dj)*8 + c, free dim = padded l' = ho*64+wo."""
    ap = x_d[b, h * 8 : (h + 1) * 8, ho0 + di, 0:3].unsqueeze(-1)
    v = ap.ap
    v[0] = [1, 3]
    v[1] = [H * WIDTH, 8]
    v[2] = [1, lt_load]
    return ap


def _ldw_signature(inst):
    """Identity of a Ldweights' stationary operand."""
    return str(inst.ins[0])


def _dedupe_ldweights(nc):
    """Remove Ldweights whose stationary operand is identical to the still-
    loaded one (only Matmults / non-PE instructions in between).  Waits and
    updates of a removed Ldweights move to the following PE instruction."""
    pe = mybir.EngineType.PE
    removed = 0
    for blk in nc.m.functions[0].blocks:
        last_sig = None
        insts = blk.instructions
        idx = 0
        while idx < len(insts):
            inst = insts[idx]
            if getattr(inst, "engine", None) != pe:
                idx += 1
                continue
            op = str(inst.opcode)
            if op == "Ldweights":
                sig = _ldw_signature(inst)
                if sig is not None and sig == last_sig:
                    nxt = None
                    for j in range(idx + 1, len(insts)):
                        if getattr(insts[j], "engine", None) == pe:
                            nxt = insts[j]
                            break
                    si = inst.sync_info
                    has_sync = si is not None and (
                        len(si.on_wait) > 0 or len(si.on_update) > 0
                    )
                    if nxt is not None:
                        nxt.merge_dependencies_from(inst)
                        if has_sync:
                            nsi = nxt.sync_info
                            if nsi is None:
                                nxt.sync_info = si
                            else:
                                for w in si.on_wait:
                                    nsi.on_wait.append(w)
                                for u in si.on_update:
                                    nsi.on_update.append(u)
                        del insts[idx]
                        removed += 1
                        continue
                last_sig = sig
            elif op != "Matmult":
                if op not in ("EventSemaphore", "Nop"):
                    last_sig = None
            idx += 1
    return removed


def build_nc(
    reps: int = 1,
    skew: int = 1,
    py_reps: int = 1,
    dedupe: bool = True,
    sel_group: int = 2,
    tail_split: bool = True,
    defer_copy: bool = False,
    alt_store: bool = True,
    end_taper: int = 0,
    compact: bool = False,
    alt_drains: bool = False,
    dve_w: float = 1.79,
    tmp_bufs: int = 6,
    obuf_bufs: int = 8,
):
    """Build the per-core program.  reps>1 wraps the body in an on-chip loop
    (used only for device-time measurement); skew is the software-pipeline
    depth between a tile's selection/squares and its main matmuls;
    sel_group is the number of PSUM banks drained by one square op."""
    f32, f16 = mybir.dt.float32, mybir.dt.float16
    nc = bacc.Bacc("TRN2", target_bir_lowering=False)
    x_d = nc.dram_tensor("x_loc", [B_LOC, C, H, WIDTH], f16, kind="ExternalInput")
    a_d = nc.dram_tensor("aselT", [72, GH], f16, kind="ExternalInput")
    w_d = nc.dram_tensor("w2T", [2 * GH, O], f16, kind="ExternalInput")
    o_d = nc.dram_tensor("out_loc", [B_LOC, O, HO, WO], f32, kind="ExternalOutput")

    ndrain = NCHUNK // sel_group  # square ops per tile
    sel_bufs = 6 // sel_group  # keep 6 PSUM banks for selections

    with TileContext(nc) as tc:
        with (
            tc.tile_pool(name="const", bufs=1) as cpool,
            tc.tile_pool(name="xin", bufs=2) as xpool,
            tc.tile_pool(name="gbuf", bufs=ndrain * (skew + 1) + ndrain) as gpool,
            tc.tile_pool(name="tmpbuf", bufs=tmp_bufs) as tmppool,
            tc.tile_pool(name="obuf", bufs=obuf_bufs) as opool,
            tc.tile_pool(name="ps_sel", bufs=sel_bufs, space="PSUM") as pspool,
            tc.tile_pool(name="ps_out", bufs=2, space="PSUM") as popool,
        ):
            if compact:
                LFULL = HO * WO  # 3844 dense l = ho*62+wo columns
                TILES = [(k * 512, 512) for k in range(7)] + [(3584, 260)]
            else:
                LFULL = HO * 64  # 3968 columns of the padded l' layout
                TILES = [(ho0 * 64, nr * 64) for ho0, nr in ROW_TILES]

            a_r = cpool.tile([72, GH], f16, tag="a_r")
            nc.sync.dma_start(a_r[:], a_d[:])

            def load_x(x_t, b, h, col0, col1, eng=None):
                """Fill x_t[:, col0:col1] of the unfold view for (b, c-half h)."""
                eng = eng or nc.sync
                for di in range(3):
                    hi = min(col1, H * WIDTH - di * 64 - 2)
                    if hi > col0:
                        ap = _x_window_ap(x_d, b, h, 0, di, hi - col0)
                        ap.offset += col0
                        eng.dma_start(x_t[di * 24 : (di + 1) * 24, col0:hi], ap)
                    if hi < col1:
                        # pad columns feed discarded outputs; fill with
                        # arbitrary valid f32r data to keep reads clean
                        eng.dma_start(
                            x_t[di * 24 : (di + 1) * 24, hi:col1],
                            _x_window_ap(x_d, b, h, 0, 0, col1 - hi),
                        )

            def load_x_compact(x_t, b, h, ho0, ho1, eng=None):
                """Fill x_t[:, ho0*62:ho1*62] of the dense unfold view.
                One DMA per kernel offset (di, dj): 3-dim APs."""
                eng = eng or nc.sync
                nrow = ho1 - ho0
                for di in range(3):
                    for dj in range(3):
                        ap = x_d[b, h * 8 : (h + 1) * 8, di, dj : dj + 1]
                        ap = ap.unsqueeze(-1)
                        v = ap.ap
                        v[0] = [H * WIDTH, 8]  # channel
                        v[1] = [WIDTH, nrow]   # ho rows
                        v[2] = [1, WO]         # wo
                        ap.offset += ho0 * WIDTH
                        r0 = (di * 3 + dj) * 8
                        eng.dma_start(
                            x_t[r0 : r0 + 8, ho0 * WO : ho1 * WO], ap
                        )

            # all unfold loads up front; batch 0 split so tile 0 starts early
            xr_all = []
            for b in range(B_LOC):
                xr_b = []
                for h in range(2):
                    x_t = xpool.tile([72, LFULL], f16, tag=f"x{h}", name=f"x{h}_{b}")
                    xr_b.append(x_t)
                xr_all.append(xr_b)
            if compact:
                for h in range(2):
                    load_x_compact(xr_all[0][h], 0, h, 0, 17)
            else:
                for h in range(2):
                    load_x(xr_all[0][h], 0, h, 0, 1024)
            w_r = cpool.tile([GC, NCHUNK, O], f16, tag="w_r")
            nc.sync.dma_start(w_r[:], w_d[:].rearrange("(k p) o -> p k o", p=GC))
            if compact:
                for h in range(2):
                    load_x_compact(xr_all[0][h], 0, h, 17, HO)
                for b in range(1, B_LOC):
                    for h in range(2):
                        load_x_compact(xr_all[b][h], b, h, 0, HO)
            else:
                for h in range(2):
                    load_x(xr_all[0][h], 0, h, 1024, LFULL)
                for b in range(1, B_LOC):
                    for h in range(2):
                        load_x(xr_all[b][h], b, h, 0, LFULL)

            # greedy ACT/DVE load balancing for PSUM-draining elementwise
            # ops; weights are modeled engine-ns for the op shapes in use
            eng_busy = {"act": 0.0, "dve": 0.0}
            SQ_W = {2: (1.04, dve_w), 3: (1.47, 2.59)}[sel_group]

            def square_merged(g_t, ps_s, lt, kp=None):
                gv = g_t[:, :, :lt]
                pv = ps_s[:, :, :lt]
                if alt_drains and kp is not None and kp < 2:
                    use_act = kp == 0  # first drain fast-latency on ACT
                else:
                    use_act = eng_busy["act"] + SQ_W[0] <= eng_busy["dve"] + SQ_W[1]
                if use_act:
                    nc.scalar.square(gv, pv)
                    eng_busy["act"] += SQ_W[0]
                else:
                    tmp = tmppool.tile([GC, sel_group, 512], f16, tag="sq_tmp")
                    tv = tmp[:, :, :lt]
                    nc.vector.tensor_copy(tv, pv)
                    nc.vector.tensor_mul(gv, tv, tv)
                    eng_busy["dve"] += SQ_W[1]

            def out_copy(o_view, ps_view, force=None):
                if force == "act" or (
                    force is None
                    and eng_busy["act"] + 0.60 < eng_busy["dve"] + 0.64
                ):
                    nc.scalar.copy(o_view, ps_view)
                    eng_busy["act"] += 0.60
                else:
                    nc.vector.tensor_copy(o_view, ps_view)
                    eng_busy["dve"] += 0.64

            copy_queue = []
            store_ctr = {"n": 0}

            def _out_flat_ap(b, c0, lt):
                """Flat [O, lt] view of out_loc at dense column offset c0."""
                ap = o_d[b, :, 0, 0:2].unsqueeze(-1)
                v = ap.ap
                v[1] = [1, lt]
                v[2] = [1, 1]
                ap.offset += c0
                return ap

            def drain_tile(item, last=False):
                """Drain one accumulator to SBUF and store it."""
                b, c0, lt, ps_o = item
                eng = nc.gpsimd
                if alt_store:
                    store_ctr["n"] += 1
                    if store_ctr["n"] % 2 == 0 or last:
                        eng = nc.sync
                if compact:
                    o_t = opool.tile([O, 512], f32, tag="o", name="o_t")
                    if last and tail_split and lt >= 128:
                        half = lt // 2
                        out_copy(o_t[:, :half], ps_o[:, :half], force="act")
                        out_copy(o_t[:, half:lt], ps_o[:, half:lt], force="dve")
                    else:
                        out_copy(o_t[:, :lt], ps_o[:, :lt])
                    eng.dma_start(_out_flat_ap(b, c0, lt), o_t[:, :lt])
                else:
                    ho0, nr = c0 // 64, lt // 64
                    o_t = opool.tile([O, 8 * WO], f32, tag="o", name="o_t")
                    ps_view = ps_o[:, :lt].rearrange("o (r w) -> o r w", w=64)
                    o_view = o_t[:, : nr * WO].rearrange("o (r w) -> o r w", w=WO)
                    if last and tail_split and nr > 1:
                        half = nr // 2
                        out_copy(o_view[:, :half], ps_view[:, :half, :WO], force="act")
                        out_copy(o_view[:, half:], ps_view[:, half:nr, :WO], force="dve")
                    else:
                        out_copy(o_view, ps_view[:, :, :WO])
                    eng.dma_start(
                        o_d[b, :, ho0 : ho0 + nr, :],
                        o_t[:, : nr * WO],
                    )

            def do_mains(st, last=False):
                """Main matmuls + drain for a tile whose squares are issued."""
                b, c0, lt, g_ts = st
                ps_o = popool.tile([O, 512], f32, tag="ps_o", name="ps_o")
                for kk in range(NCHUNK):
                    nc.tensor.matmul(
                        ps_o[:, :lt],
                        w_r[:, kk, :],
                        g_ts[kk // sel_group][:, kk % sel_group, :lt],
                        start=(kk == 0),
                        stop=(kk == NCHUNK - 1),
                    )
                if defer_copy:
                    # emit the PSUM->SBUF drain one tile-slot later so it
                    # enqueues behind the next tile's PE-blocking squares
                    copy_queue.append((b, c0, lt, ps_o))
                    if len(copy_queue) > 1:
                        drain_tile(copy_queue.pop(0))
                    if last:
                        while copy_queue:
                            drain_tile(copy_queue.pop(0), last=not copy_queue)
                else:
                    drain_tile((b, c0, lt, ps_o), last=last)

            # HAM warmup: keep the PE busy during the initial DMA wait so the
            # clock gate is at 8/8 when real matmuls start (dummy MMs on the
            # first tile that lands; outputs never read)
            def warmup():
                for i in range(12):
                    ps_w = popool.tile([O, 512], f32, tag="ps_o", name="warm")
                    nc.tensor.matmul(
                        ps_w[:, :360], a_r[:, :128], a_r[:, :360],
                        start=True, stop=True,
                    )

            def body(it=None, unroll=1):
                # software-pipeline skew: issue tile t's selections and
                # squares, then tile (t-skew)'s mains — squares get `skew`
                # tiles of slack before the PE needs their output
                pending = []
                tiles_total = B_LOC * len(TILES)
                for b in range(B_LOC):
                    xr = xr_all[b]
                    for ti, (c0, lt) in enumerate(TILES):
                        g_ts = []
                        for kp in range(ndrain):
                            ps_s = pspool.tile(
                                [GC, sel_group, 512], f32, tag="ps_s", name="ps_s"
                            )
                            for half in range(sel_group):
                                kk = kp * sel_group + half
                                h, k = divmod(kk, 3)
                                nc.tensor.matmul(
                                    ps_s[:, half, :lt],
                                    a_r[:, k * GC : (k + 1) * GC],
                                    xr[h][:, c0 : c0 + lt],
                                    start=True,
                                    stop=True,
                                )
                            g_t = gpool.tile(
                                [GC, sel_group, 512], f16, tag="g", name="g_t"
                            )
                            square_merged(g_t, ps_s, lt, kp=kp)
                            g_ts.append(g_t)
                        pending.append((b, c0, lt, g_ts))
                        tile_idx = b * len(TILES) + ti
                        eff_skew = skew
                        if end_taper and tile_idx >= tiles_total - end_taper:
                            eff_skew = max(1, skew - (tile_idx - (tiles_total - end_taper) + 1))
                        if len(pending) > eff_skew:
                            do_mains(pending.pop(0))
                for i, st in enumerate(pending):
                    do_mains(st, last=(i == len(pending) - 1))

            warmup()
            if py_reps > 1:
                # python-unrolled loop for TimelineSim steady-state modeling
                for _ in range(py_reps):
                    body()
            elif reps == 1:
                body()
            else:
                hint = (
                    mybir.EngineType.PE,
                    mybir.EngineType.Activation,
                    mybir.EngineType.DVE,
                    mybir.EngineType.SP,
                    mybir.EngineType.Pool,
                )
                with tc.For_i(0, reps, 1, hint_engines=hint) as _it:
                    body()
    if dedupe:
        _dedupe_ldweights(nc)
    nc.compile()
    return nc


@functools.lru_cache(maxsize=1)
def _cached_nc():
    return build_nc()


def kernel(x: np.ndarray, W: np.ndarray, _trace: bool = False):
    x = np.asarray(x, dtype=np.float32)
    W = np.asarray(W, dtype=np.float32)
    AselT, W2T = _build_consts(W)
    x_r = x.astype(np.float16)

    nc = _cached_nc()
    in_maps = [
        {
            "x_loc": np.ascontiguousarray(x_r[k * B_LOC : (k + 1) * B_LOC]),
            "aselT": AselT,
            "w2T": W2T,
        }
        for k in range(N_CORES)
    ]
    try:
        r = run_bass_kernel_spmd(
            nc, in_maps, core_ids=list(range(N_CORES)), trace=_trace
        )
    except Exception:
        # transient NRT_EXEC_UNIT_UNRECOVERABLE has been observed once on
        # this fabric; a fresh attempt recovers
        r = run_bass_kernel_spmd(
            nc, in_maps, core_ids=list(range(N_CORES)), trace=_trace
        )
    out = np.concatenate([m["out_loc"] for m in r.results], axis=0)
    if _trace:
        kernel.last_result = r
    return out


if __name__ == "__main__":
    rng = np.random.default_rng(0)
    x = rng.standard_normal((B, C, H, WIDTH), dtype=np.float32)
    W = rng.standard_normal((O, C * 81), dtype=np.float32)
    out = kernel(x, W)
    print("out shape", out.shape, out.dtype)


# revision 40
# speedup vs baseline: 4.3052x; 1.0245x over previous
"""Trainium2 Bass kernel for nn_Fast2Order_DE_Conv.

Math: out[b,o,ho,wo] = sum_{c,i,j} W[o, c*81+i*9+j] * p_i * p_j with
p_i = x[b, c, ho+di, wo+dj] (i = di*3+dj, 3x3 unfold of a 16-channel 64x64
image; output 62x62).

Algorithm: change the quadratic-feature basis from products p_i*p_j to
squares {p_i^2, (p_i+p_j)^2, i<j} (45 per channel, 720 total) and fold the
basis change into W on the host (W2 = W * M^-1).  On-chip, per spatial tile
of 512 locations:

    selection matmul (PE, f16):  s = AselT.T @ x_unfold  [768 padded rows]
    square          (ACT/DVE):   g = s^2, PSUM -> SBUF f16
    main matmul     (PE, f16):   out += W2T.T @ g, accumulated in fp32 PSUM

All matmuls use float16 (e5m10: ~f32r accuracy at half the width, 2-byte
FWL-eligible weight loads, full PE rate).  Inputs are cast to f16 on the
host so DMA loads feed the PE directly.  The 3x3 unfold itself is free: it
is expressed in the DMA access pattern (overlapping windows of the padded
l' = ho*64+wo layout).

Pipelining: tiles are software-pipelined with skew 1 (a tile's selection
matmuls + squares issue one tile before its main matmuls; the shallow skew
with deep tmp/output buffering measured fastest on this fabric); a burst
of warmup matmuls during the initial DMA window keeps the PE clock gate at
full rate.

DVE squares bounce PSUM->SBUF through an f16 tmp so the multiply runs in
the DVE 2x_1p perf mode; ACT squares go straight from PSUM.  A greedy
balancer splits squares and output copies across ACT/DVE by modeled ns.
Output stores alternate between the Pool SWDGE and SP HWDGE queues so
consecutive stores issue in parallel, and the final tile's PSUM drain is
split across both elementwise engines to shorten the kernel tail.  A
post-build pass drops Ldweights whose stationary operand is already
loaded (warmup runs and back-to-back same-weight matmuls).

Sharding: data-parallel over batch, 2 batches per core on 8 cores; W-side
constants are replicated.  Output gathered by simple concatenation.
"""

import functools

import numpy as np

import concourse.bacc as bacc
import concourse.mybir as mybir
from concourse.tile import TileContext
from concourse.bass_utils import run_bass_kernel_spmd

B, C, H, WIDTH = 16, 16, 64, 64
O = 128
HO = WO = 62
N_CORES = 8
B_LOC = B // N_CORES
PAIRS = [(i, j) for i in range(9) for j in range(i, 9)]  # 45
ROW_TILES = [(0, 8), (8, 8), (16, 8), (24, 8), (32, 8), (40, 8), (48, 8), (56, 6)]
NCHUNK = 6  # g chunks of 128 rows (768 total, 48 zero-padded)
GC = 128
GH = 384  # padded g rows per c-half (360 real + 24 pad)


def _round_f32r(a: np.ndarray) -> np.ndarray:
    """Round fp32 values to the f32r grid (RNE at 12 low mantissa bits)."""
    a = np.ascontiguousarray(a, dtype=np.float32)
    bits = a.view(np.uint32).astype(np.uint64)
    half, mask = np.uint64(0x800), np.uint64(0xFFF)
    lsb = (bits >> np.uint64(12)) & np.uint64(1)
    out = ((bits + half - np.uint64(1) + lsb) & ~mask).astype(np.uint32)
    return out.view(np.float32).reshape(a.shape)


def _build_consts(Wf: np.ndarray):
    """W (128, 1296) -> (AselT [72, 384] f16, W2T [768, 128] f16)."""
    Wt = np.asarray(Wf, dtype=np.float64).reshape(O, C, 9, 9)
    Wsym = Wt + Wt.transpose(0, 1, 3, 2)
    W2 = np.zeros((O, 720))
    for c in range(C):
        for pi, (i, j) in enumerate(PAIRS):
            f = c * 45 + pi
            if i == j:
                W2[:, f] = Wt[:, c, i, i] - 0.5 * (
                    Wsym[:, c, i, :].sum(-1) - 2.0 * Wt[:, c, i, i]
                )
            else:
                W2[:, f] = 0.5 * Wsym[:, c, i, j]
    # x-row layout on chip: row = i*8 + c_local (i = di*3+dj kernel position)
    AselT = np.zeros((72, 384), dtype=np.float32)
    for cl in range(8):
        for pi, (i, j) in enumerate(PAIRS):
            g = cl * 45 + pi
            AselT[i * 8 + cl, g] += 1.0
            if i != j:
                AselT[j * 8 + cl, g] += 1.0
    # pad each c-half's 360 features to 384 (3 chunks of 128) so every
    # selection matmul has exactly 128 stationary columns
    W2p = np.zeros((O, 768))
    W2p[:, 0:360] = W2[:, 0:360]
    W2p[:, 384:744] = W2[:, 360:720]
    W2T = np.ascontiguousarray(W2p.T).astype(np.float16)  # [768, 128]
    return AselT.astype(np.float16), W2T


def _x_window_ap(x_d, b: int, h: int, ho0: int, di: int, lt_load: int):
    """Source AP for one di of the unfold load: (dj, c, l) nesting matching
    target partitions (di*3+dj)*8 + c, free dim = padded l' = ho*64+wo."""
    ap = x_d[b, h * 8 : (h + 1) * 8, ho0 + di, 0:3].unsqueeze(-1)
    v = ap.ap
    v[0] = [1, 3]
    v[1] = [H * WIDTH, 8]
    v[2] = [1, lt_load]
    return ap


def _ldw_signature(inst):
    """Identity of a Ldweights' stationary operand."""
    return str(inst.ins[0])


def _dedupe_ldweights(nc):
    """Remove Ldweights whose stationary operand is identical to the still-
    loaded one (only Matmults / non-PE instructions in between).  Waits and
    updates of a removed Ldweights move to the following PE instruction."""
    pe = mybir.EngineType.PE
    removed = 0
    for blk in nc.m.functions[0].blocks:
        last_sig = None
        insts = blk.instructions
        idx = 0
        while idx < len(insts):
            inst = insts[idx]
            if getattr(inst, "engine", None) != pe:
                idx += 1
                continue
            op = str(inst.opcode)
            if op == "Ldweights":
                sig = _ldw_signature(inst)
                if sig is not None and sig == last_sig:
                    nxt = None
                    for j in range(idx + 1, len(insts)):
                        if getattr(insts[j], "engine", None) == pe:
                            nxt = insts[j]
                            break
                    si = inst.sync_info
                    has_sync = si is not None and (
                        len(si.on_wait) > 0 or len(si.on_update) > 0
                    )
                    if nxt is not None:
                        nxt.merge_dependencies_from(inst)
                        if has_sync:
                            nsi = nxt.sync_info
                            if nsi is None:
                                nxt.sync_info = si
                            else:
                                for w in si.on_wait:
                                    nsi.on_wait.append(w)
                                for u in si.on_update:
                                    nsi.on_update.append(u)
                        del insts[idx]
                        removed += 1
                        continue
                last_sig = sig
            elif op != "Matmult":
                if op not in ("EventSemaphore", "Nop"):
                    last_sig = None
            idx += 1
    return removed


def build_nc(
    reps: int = 1,
    skew: int = 1,
    py_reps: int = 1,
    dedupe: bool = True,
    sel_group: int = 2,
    tail_split: bool = True,
    defer_copy: bool = False,
    alt_store: bool = True,
    end_taper: int = 0,
    compact: bool = False,
    alt_drains: bool = False,
    dve_w: float = 1.79,
    tmp_bufs: int = 6,
    obuf_bufs: int = 8,
    staggered: bool = False,
):
    """Build the per-core program.  reps>1 wraps the body in an on-chip loop
    (used only for device-time measurement); skew is the software-pipeline
    depth between a tile's selection/squares and its main matmuls;
    sel_group is the number of PSUM banks drained by one square op."""
    f32, f16 = mybir.dt.float32, mybir.dt.float16
    nc = bacc.Bacc("TRN2", target_bir_lowering=False)
    x_d = nc.dram_tensor("x_loc", [B_LOC, C, H, WIDTH], f16, kind="ExternalInput")
    a_d = nc.dram_tensor("aselT", [72, GH], f16, kind="ExternalInput")
    w_d = nc.dram_tensor("w2T", [2 * GH, O], f16, kind="ExternalInput")
    o_d = nc.dram_tensor("out_loc", [B_LOC, O, HO, WO], f32, kind="ExternalOutput")

    ndrain = NCHUNK // sel_group  # square ops per tile
    sel_bufs = 6 // sel_group  # keep 6 PSUM banks for selections

    with TileContext(nc) as tc:
        with (
            tc.tile_pool(name="const", bufs=1) as cpool,
            tc.tile_pool(name="xin", bufs=2) as xpool,
            tc.tile_pool(name="gbuf", bufs=ndrain * (skew + 1) + ndrain) as gpool,
            tc.tile_pool(name="tmpbuf", bufs=tmp_bufs) as tmppool,
            tc.tile_pool(name="obuf", bufs=obuf_bufs) as opool,
            tc.tile_pool(name="ps_sel", bufs=sel_bufs, space="PSUM") as pspool,
            tc.tile_pool(name="ps_out", bufs=2, space="PSUM") as popool,
        ):
            if compact:
                LFULL = HO * WO  # 3844 dense l = ho*62+wo columns
                TILES = [(k * 512, 512) for k in range(7)] + [(3584, 260)]
            else:
                LFULL = HO * 64  # 3968 columns of the padded l' layout
                TILES = [(ho0 * 64, nr * 64) for ho0, nr in ROW_TILES]

            a_r = cpool.tile([72, GH], f16, tag="a_r")
            nc.sync.dma_start(a_r[:], a_d[:])

            def load_x(x_t, b, h, col0, col1, eng=None):
                """Fill x_t[:, col0:col1] of the unfold view for (b, c-half h)."""
                eng = eng or nc.sync
                for di in range(3):
                    hi = min(col1, H * WIDTH - di * 64 - 2)
                    if hi > col0:
                        ap = _x_window_ap(x_d, b, h, 0, di, hi - col0)
                        ap.offset += col0
                        eng.dma_start(x_t[di * 24 : (di + 1) * 24, col0:hi], ap)
                    if hi < col1:
                        # pad columns feed discarded outputs; fill with
                        # arbitrary valid f32r data to keep reads clean
                        eng.dma_start(
                            x_t[di * 24 : (di + 1) * 24, hi:col1],
                            _x_window_ap(x_d, b, h, 0, 0, col1 - hi),
                        )

            def load_x_compact(x_t, b, h, ho0, ho1, eng=None):
                """Fill x_t[:, ho0*62:ho1*62] of the dense unfold view.
                One DMA per kernel offset (di, dj): 3-dim APs."""
                eng = eng or nc.sync
                nrow = ho1 - ho0
                for di in range(3):
                    for dj in range(3):
                        ap = x_d[b, h * 8 : (h + 1) * 8, di, dj : dj + 1]
                        ap = ap.unsqueeze(-1)
                        v = ap.ap
                        v[0] = [H * WIDTH, 8]  # channel
                        v[1] = [WIDTH, nrow]   # ho rows
                        v[2] = [1, WO]         # wo
                        ap.offset += ho0 * WIDTH
                        r0 = (di * 3 + dj) * 8
                        eng.dma_start(
                            x_t[r0 : r0 + 8, ho0 * WO : ho1 * WO], ap
                        )

            # all unfold loads up front; batch 0 split so tile 0 starts early
            xr_all = []
            for b in range(B_LOC):
                xr_b = []
                for h in range(2):
                    x_t = xpool.tile([72, LFULL], f16, tag=f"x{h}", name=f"x{h}_{b}")
                    xr_b.append(x_t)
                xr_all.append(xr_b)
            if compact:
                for h in range(2):
                    load_x_compact(xr_all[0][h], 0, h, 0, 17)
            else:
                for h in range(2):
                    load_x(xr_all[0][h], 0, h, 0, 1024)
            w_r = cpool.tile([GC, NCHUNK, O], f16, tag="w_r")
            nc.sync.dma_start(w_r[:], w_d[:].rearrange("(k p) o -> p k o", p=GC))
            if compact:
                for h in range(2):
                    load_x_compact(xr_all[0][h], 0, h, 17, HO)
                for b in range(1, B_LOC):
                    for h in range(2):
                        load_x_compact(xr_all[b][h], b, h, 0, HO)
            else:
                for h in range(2):
                    load_x(xr_all[0][h], 0, h, 1024, LFULL)
                for b in range(1, B_LOC):
                    for h in range(2):
                        load_x(xr_all[b][h], b, h, 0, LFULL)

            # greedy ACT/DVE load balancing for PSUM-draining elementwise
            # ops; weights are modeled engine-ns for the op shapes in use
            eng_busy = {"act": 0.0, "dve": 0.0}
            SQ_W = {2: (1.04, dve_w), 3: (1.47, 2.59)}[sel_group]

            def square_merged(g_t, ps_s, lt, kp=None):
                gv = g_t[:, :, :lt]
                pv = ps_s[:, :, :lt]
                if alt_drains and kp is not None and kp < 2:
                    use_act = kp == 0  # first drain fast-latency on ACT
                else:
                    use_act = eng_busy["act"] + SQ_W[0] <= eng_busy["dve"] + SQ_W[1]
                if use_act:
                    nc.scalar.square(gv, pv)
                    eng_busy["act"] += SQ_W[0]
                else:
                    tmp = tmppool.tile([GC, sel_group, 512], f16, tag="sq_tmp")
                    tv = tmp[:, :, :lt]
                    nc.vector.tensor_copy(tv, pv)
                    nc.vector.tensor_mul(gv, tv, tv)
                    eng_busy["dve"] += SQ_W[1]

            def out_copy(o_view, ps_view, force=None):
                if force == "act" or (
                    force is None
                    and eng_busy["act"] + 0.60 < eng_busy["dve"] + 0.64
                ):
                    nc.scalar.copy(o_view, ps_view)
                    eng_busy["act"] += 0.60
                else:
                    nc.vector.tensor_copy(o_view, ps_view)
                    eng_busy["dve"] += 0.64

            copy_queue = []
            store_ctr = {"n": 0}

            def _out_flat_ap(b, c0, lt):
                """Flat [O, lt] view of out_loc at dense column offset c0."""
                ap = o_d[b, :, 0, 0:2].unsqueeze(-1)
                v = ap.ap
                v[1] = [1, lt]
                v[2] = [1, 1]
                ap.offset += c0
                return ap

            def drain_tile(item, last=False):
                """Drain one accumulator to SBUF and store it."""
                b, c0, lt, ps_o = item
                eng = nc.gpsimd
                if alt_store == "sync":
                    eng = nc.sync
                elif alt_store:
                    store_ctr["n"] += 1
                    if store_ctr["n"] % 2 == 0 or last:
                        eng = nc.sync
                if compact:
                    o_t = opool.tile([O, 512], f32, tag="o", name="o_t")
                    if last and tail_split and lt >= 128:
                        half = lt // 2
                        out_copy(o_t[:, :half], ps_o[:, :half], force="act")
                        out_copy(o_t[:, half:lt], ps_o[:, half:lt], force="dve")
                    else:
                        out_copy(o_t[:, :lt], ps_o[:, :lt])
                    eng.dma_start(_out_flat_ap(b, c0, lt), o_t[:, :lt])
                else:
                    ho0, nr = c0 // 64, lt // 64
                    o_t = opool.tile([O, 8 * WO], f32, tag="o", name="o_t")
                    ps_view = ps_o[:, :lt].rearrange("o (r w) -> o r w", w=64)
                    o_view = o_t[:, : nr * WO].rearrange("o (r w) -> o r w", w=WO)
                    if last and tail_split and nr > 1:
                        half = nr // 2
                        out_copy(o_view[:, :half], ps_view[:, :half, :WO], force="act")
                        out_copy(o_view[:, half:], ps_view[:, half:nr, :WO], force="dve")
                    else:
                        out_copy(o_view, ps_view[:, :, :WO])
                    eng.dma_start(
                        o_d[b, :, ho0 : ho0 + nr, :],
                        o_t[:, : nr * WO],
                    )

            def do_mains(st, last=False):
                """Main matmuls + drain for a tile whose squares are issued."""
                b, c0, lt, g_ts = st
                ps_o = popool.tile([O, 512], f32, tag="ps_o", name="ps_o")
                for kk in range(NCHUNK):
                    nc.tensor.matmul(
                        ps_o[:, :lt],
                        w_r[:, kk, :],
                        g_ts[kk // sel_group][:, kk % sel_group, :lt],
                        start=(kk == 0),
                        stop=(kk == NCHUNK - 1),
                    )
                if defer_copy:
                    # emit the PSUM->SBUF drain one tile-slot later so it
                    # enqueues behind the next tile's PE-blocking squares
                    copy_queue.append((b, c0, lt, ps_o))
                    if len(copy_queue) > 1:
                        drain_tile(copy_queue.pop(0))
                    if last:
                        while copy_queue:
                            drain_tile(copy_queue.pop(0), last=not copy_queue)
                else:
                    drain_tile((b, c0, lt, ps_o), last=last)

            # HAM warmup: keep the PE busy during the initial DMA wait so the
            # clock gate is at 8/8 when real matmuls start (dummy MMs on the
            # first tile that lands; outputs never read)
            def warmup():
                for i in range(12):
                    ps_w = popool.tile([O, 512], f32, tag="ps_o", name="warm")
                    nc.tensor.matmul(
                        ps_w[:, :360], a_r[:, :128], a_r[:, :360],
                        start=True, stop=True,
                    )

            def body(it=None, unroll=1):
                # software-pipeline skew: issue tile t's selections and
                # squares, then tile (t-skew)'s mains — squares get `skew`
                # tiles of slack before the PE needs their output
                pending = []
                tiles_total = B_LOC * len(TILES)
                for b in range(B_LOC):
                    xr = xr_all[b]
                    for ti, (c0, lt) in enumerate(TILES):
                        g_ts = []
                        for kp in range(ndrain):
                            ps_s = pspool.tile(
                                [GC, sel_group, 512], f32, tag="ps_s", name="ps_s"
                            )
                            for half in range(sel_group):
                                kk = kp * sel_group + half
                                h, k = divmod(kk, 3)
                                nc.tensor.matmul(
                                    ps_s[:, half, :lt],
                                    a_r[:, k * GC : (k + 1) * GC],
                                    xr[h][:, c0 : c0 + lt],
                                    start=True,
                                    stop=True,
                                )
                            g_t = gpool.tile(
                                [GC, sel_group, 512], f16, tag="g", name="g_t"
                            )
                            square_merged(g_t, ps_s, lt, kp=kp)
                            g_ts.append(g_t)
                        pending.append((b, c0, lt, g_ts))
                        tile_idx = b * len(TILES) + ti
                        eff_skew = skew
                        if end_taper and tile_idx >= tiles_total - end_taper:
                            eff_skew = max(1, skew - (tile_idx - (tiles_total - end_taper) + 1))
                        if len(pending) > eff_skew:
                            do_mains(pending.pop(0))
                for i, st in enumerate(pending):
                    do_mains(st, last=(i == len(pending) - 1))

            warmup()
            if py_reps > 1:
                # python-unrolled loop for TimelineSim steady-state modeling
                for _ in range(py_reps):
                    body()
            elif reps == 1:
                body()
            else:
                hint = (
                    mybir.EngineType.PE,
                    mybir.EngineType.Activation,
                    mybir.EngineType.DVE,
                    mybir.EngineType.SP,
                    mybir.EngineType.Pool,
                )
                with tc.For_i(
                    0, reps, 1, hint_engines=hint, staggered_reset=staggered
                ) as _it:
                    body()
    if dedupe:
        _dedupe_ldweights(nc)
    nc.compile()
    return nc


@functools.lru_cache(maxsize=1)
def _cached_nc():
    return build_nc()


def kernel(x: np.ndarray, W: np.ndarray, _trace: bool = False):
    x = np.asarray(x, dtype=np.float32)
    W = np.asarray(W, dtype=np.float32)
    AselT, W2T = _build_consts(W)
    x_r = x.astype(np.float16)

    nc = _cached_nc()
    in_maps = [
        {
            "x_loc": np.ascontiguousarray(x_r[k * B_LOC : (k + 1) * B_LOC]),
            "aselT": AselT,
            "w2T": W2T,
        }
        for k in range(N_CORES)
    ]
    try:
        r = run_bass_kernel_spmd(
            nc, in_maps, core_ids=list(range(N_CORES)), trace=_trace
        )
    except Exception:
        # transient NRT_EXEC_UNIT_UNRECOVERABLE has been observed once on
        # this fabric; a fresh attempt recovers
        r = run_bass_kernel_spmd(
            nc, in_maps, core_ids=list(range(N_CORES)), trace=_trace
        )
    out = np.concatenate([m["out_loc"] for m in r.results], axis=0)
    if _trace:
        kernel.last_result = r
    return out


if __name__ == "__main__":
    rng = np.random.default_rng(0)
    x = rng.standard_normal((B, C, H, WIDTH), dtype=np.float32)
    W = rng.standard_normal((O, C * 81), dtype=np.float32)
    out = kernel(x, W)
    print("out shape", out.shape, out.dtype)
